# revision 12
# baseline (speedup 1.0000x reference)
"""Dense dot-product attention on 8 Trainium2 NeuronCores.

Problem: query/key/value [32, 2048, 64] fp32 -> softmax(Q K^T / 8) V.
Sharding: batch dim split 4-per-core across 8 cores (data parallel, no
collectives). Each core computes full attention for its 4 batches.

Design (ScalarE-exp-bound, PE kept below the exp floor):
  - All matmuls fp16 (1 cycle/col on the PE; keeps HAM at 2.4 GHz).
  - Q/K transposed to [d, seq] layout by the DMA xbar transpose engine
    (fp16, hidden under compute) -- zero PE transposes. K lands
    pair-packed: khT2[:, p, :] holds K^T tile 2p in partitions 0-63 and
    tile 2p+1 in partitions 64-127, feeding the two 64-row S-matmul
    strips directly. Q is reshuffled into a flat [64, 2048] layout and
    duplicated into both partition halves.
  - S^T[k, q] accumulates into a single persistent PSUM tile
    [128, 3072] = three rotating 1024-col k-tile buffers. exp runs on
    ScalarE over two adjacent buffers at once (N=2048) whenever the
    rotation allows: ~75% of columns go through N=2048 activations,
    cutting the fixed ~350-cycle per-instruction overhead.
  - PV uses the stationary-streaming swap: the exp'd score chunk
    [128k, 128q] is the weight (LoadStationary, FWL at fp16), and
    [V | ones] [128k, 65] streams through. Output accumulates q-MAJOR
    [128q, 65] in PSUM: no output transpose at all, and column 64 is
    the softmax denominator for free. Weight loads double-buffer
    against the streaming matmuls, sustaining ~55-70 ns per 128x128
    weight tile.
  - Normalize: reciprocal of column 64 + per-q-block scalar multiply on
    the DVE, then DMA out.
  - Software pipeline: each plan slot issues S-matmuls for tile group
    i, the exp for group i, then PV matmuls for group i-1, so the PE
    streams while ScalarE exps and vice versa.
"""

import numpy as np

B, L, D = 32, 2048, 64
NCORES = 8
B_SH = B // NCORES          # 4 batches per core
LT = L // 128               # 16 k-tiles of 128
NQH = 2                     # q processed in halves of 1024
QHW = L // NQH              # 1024
NBLK = QHW // 128           # 8 q-blocks of 128 per qh
SCALE = 1.0 / np.sqrt(np.float32(D))  # 0.125

_cached = {}


def _build():
    import concourse.bacc as bacc
    import concourse.tile as tile
    from concourse import mybir

    f32 = mybir.dt.float32
    fp16 = mybir.dt.float16
    Exp = mybir.ActivationFunctionType.Exp

    nc = bacc.Bacc("TRN2", target_bir_lowering=False, debug=False)

    q_d = nc.dram_tensor("query", [B_SH, L, D], f32, kind="ExternalInput")
    k_d = nc.dram_tensor("key", [B_SH, L, D], f32, kind="ExternalInput")
    v_d = nc.dram_tensor("value", [B_SH, L, D], f32, kind="ExternalInput")
    o_d = nc.dram_tensor("out", [B_SH, L, D], f32, kind="ExternalOutput")

    with tile.TileContext(nc) as tc:
        with (
            tc.tile_pool(name="consts", bufs=1) as consts,
            tc.tile_pool(name="nat", bufs=2) as nat,
            tc.tile_pool(name="nath", bufs=2) as nath,
            tc.tile_pool(name="vst", bufs=2) as vst,
            tc.tile_pool(name="qkt", bufs=2) as qkt,
            tc.tile_pool(name="vr", bufs=2) as vrp,
            tc.tile_pool(name="er", bufs=2) as erp,
            tc.tile_pool(name="pvh", bufs=2) as pvhp,
            tc.tile_pool(name="ot", bufs=2) as otp,
            tc.tile_pool(name="of32", bufs=2) as of32p,
            tc.tile_pool(name="rz", bufs=8) as rzp,
            tc.tile_pool(name="sps", bufs=3, space="PSUM") as sps,
            tc.tile_pool(name="pvps", bufs=1, space="PSUM") as pvps,
        ):
            # ACT table load + warmers first: they hide under the
            # initial DMA latency and push HAM to 2.4 GHz.
            wsrc = consts.tile([128, 512], fp16)
            nc.vector.memset(wsrc, 1.0)
            dummy = consts.tile([128, 1], f32)
            nc.vector.memset(dummy, 0.0)
            nc.scalar.activation(out=dummy, in_=dummy, func=Exp, scale=1.0)

            warm_ps = pvps.tile([D + 1, QHW], f32, tag="pv")

            def warmer(n=512):
                nc.tensor.matmul(warm_ps[0:64, 0:n], wsrc[:, 0:64],
                                 wsrc[:, 0:n], start=True, stop=True,
                                 skip_group_check=True)

            # per-batch persistent tiles
            tiles = {}  # b -> (qhT, khT2, vr)

            def prep_load(b):
                """DMA loads + fp16 casts + xbar transposes for batch b.

                Returns a list of thunks (woven into the previous
                batch's stream). Dependencies are tracked by Tile, so
                placement only affects issue order.
                """
                q_nat = nat.tile([128, LT, D], f32, tag="qnat")
                k_nat = nat.tile([128, LT, D], f32, tag="knat")
                qh_nat = nath.tile([128, LT, D], fp16, tag="qh_nat")
                kh_nat = nath.tile([128, LT, D], fp16, tag="kh_nat")
                qhT = qkt.tile([128, L], fp16, tag="qhT")
                khT3 = qkt.tile([128, LT, 128], fp16, tag="khT3")
                kstage = qkt.tile([128, LT // 2, 128], fp16, tag="kstage")
                qstage = qkt.tile([128, LT // 2, 128], fp16, tag="qstage")
                v_stage = vst.tile([128, LT, D], f32, tag="vstage")
                vr = vrp.tile([128, LT, D + 1], fp16, tag="vr")

                q_r = q_d.ap()[b].rearrange("(t p) d -> p t d", p=128)
                k_r = k_d.ap()[b].rearrange("(t p) d -> p t d", p=128)

                jobs = []

                def j(fn):
                    jobs.append(fn)

                # loads (k first: S-matmul weights come from K)
                j(lambda: nc.sync.dma_start(out=k_nat[:, 0:4, :], in_=k_r[:, 0:4, :]))
                j(lambda: nc.sync.dma_start(out=q_nat[:, 0:8, :], in_=q_r[:, 0:8, :]))
                j(lambda: nc.sync.dma_start(out=k_nat[:, 4:LT, :], in_=k_r[:, 4:LT, :]))
                j(lambda: nc.sync.dma_start(out=q_nat[:, 8:LT, :], in_=q_r[:, 8:LT, :]))
                j(lambda: nc.sync.dma_start(
                    out=v_stage, in_=v_d.ap()[b].rearrange("(t p) d -> p t d", p=128)))
                # casts
                j(lambda: nc.vector.tensor_copy(out=kh_nat[:, 0:4, :], in_=k_nat[:, 0:4, :]))
                j(lambda: nc.vector.tensor_copy(out=qh_nat[:, 0:8, :], in_=q_nat[:, 0:8, :]))
                j(lambda: nc.vector.tensor_copy(out=kh_nat[:, 4:LT, :], in_=k_nat[:, 4:LT, :]))
                j(lambda: nc.vector.tensor_copy(out=qh_nat[:, 8:LT, :], in_=q_nat[:, 8:LT, :]))
                # xbar pair-transposes: [128, 2, 64] -> [128, 128] with
                # tile 2p in partitions 0-63, tile 2p+1 in 64-127.
                for p in range(LT // 2):
                    def ktr(p=p):
                        nc.sync.dma_start_transpose(
                            out=kstage[:, p, :], in_=kh_nat[:, 2 * p:2 * p + 2, :])
                    j(ktr)
                # reshuffle K^T pairs into per-tile 128-col blocks:
                # even tile t in partitions 0-63 of block t, odd tile in
                # partitions 64-127; the complementary half of each
                # block is ZERO so the S matmul can run with a full
                # C=128 contraction (128x128 tile mode -- the HAM clock
                # monitor ignores row-tiled matmuls, and a 64-row-mode
                # kernel is stuck at 1.2 GHz).
                khT3_v = khT3.rearrange("p (t e) c -> p t e c", e=2)
                if b < 2:
                    # the zero halves are only ever written by these
                    # memsets; buffers rotate 2-deep so batches 2,3
                    # reuse batch 0,1's zeros.
                    j(lambda: nc.vector.memset(khT3_v[64:128, :, 0, :], 0.0))
                    j(lambda: nc.vector.memset(khT3_v[0:64, :, 1, :], 0.0))

                def kshuf_even():
                    nc.sync.dma_start(
                        out=khT3_v[0:64, :, 0, :], in_=kstage[0:64, :, :])

                def kshuf_odd():
                    nc.sync.dma_start(
                        out=khT3_v[64:128, :, 1, :], in_=kstage[64:128, :, :])

                j(kshuf_even)
                j(kshuf_odd)
                for p in range(LT // 2):
                    def qtr(p=p):
                        nc.sync.dma_start_transpose(
                            out=qstage[:, p, :], in_=qh_nat[:, 2 * p:2 * p + 2, :])
                    j(qtr)
                # reshuffle Q^T pairs into flat [64, 2048], then dup to
                # the upper partition half for the strip-b matmuls.
                qhT_t = qhT[0:64, :].rearrange("p (t e c) -> p t e c", e=2, c=128)

                def reshuf_even():
                    nc.sync.dma_start(out=qhT_t[:, :, 0, :], in_=qstage[0:64, :, :])

                def reshuf_odd():
                    nc.sync.dma_start(out=qhT_t[:, :, 1, :], in_=qstage[64:128, :, :])

                j(reshuf_even)
                j(reshuf_odd)
                j(lambda: nc.sync.dma_start(out=qhT[64:128, :], in_=qhT[0:64, :]))
                # V: cast into [V | ones]
                j(lambda: nc.vector.tensor_copy(out=vr[:, :, 0:D], in_=v_stage))
                j(lambda: nc.vector.memset(vr[:, :, D:D + 1], 1.0))

                tiles[b] = (qhT, khT3, vr)
                return jobs

            pending = []   # deferred qh-finish jobs woven into later slots

            state = {"g": 0}  # global k-tile counter (phase for buffers)

            def main(b, next_jobs):
                qhT, khT3, vr = tiles.pop(b)
                slot = 0

                def weave(n):
                    nonlocal slot
                    if pending:
                        pending.pop(0)()
                    for _ in range(n):
                        if slot < len(next_jobs):
                            next_jobs[slot]()
                            slot += 1

                def s_mms(s, t, q0):
                    w = khT3[:, t, :]
                    for jj in range(QHW // 512):
                        nc.tensor.matmul(
                            s[:, jj * 512:(jj + 1) * 512],
                            w, qhT[:, q0 + jj * 512:q0 + (jj + 1) * 512],
                            start=True, stop=True)

                def pv_mms(pv, e_of_t):
                    for t, e_chunk in e_of_t:
                        for jj in range(QHW // 512):
                            nc.tensor.matmul(
                                pv[:, jj * 512:(jj + 1) * 512],
                                vr[:, t, :],
                                e_chunk[:, jj * 512:(jj + 1) * 512],
                                start=(t == 0), stop=(t == LT - 1))

                for qh in range(NQH):
                    q0 = qh * QHW
                    pv = pvps.tile([D + 1, QHW], f32, tag="pv")
                    prev_pv = None   # (pv, e_of_t) awaiting issue

                    for t in range(LT):
                        weave(2)
                        # own tile object per score buffer: Tile's
                        # object-granular hazard tracking then gives a
                        # 3-deep rotation (write waits on the exp from
                        # 3 tiles ago, which is long done)
                        s = sps.tile([128, QHW], f32, tag="s")
                        s_mms(s, t, q0)
                        e = erp.tile([128, QHW], fp16, tag="e")
                        nc.scalar.activation(
                            out=e, in_=s, func=Exp, scale=float(SCALE))
                        # PV for the previous tile (PE streams under
                        # the current tile's exp)
                        if prev_pv is not None:
                            pv_mms(*prev_pv)
                        prev_pv = (pv, [(t, e)])

                    pv_mms(*prev_pv)
                    state["g"] += LT

                    # deferred finish: evacuate [65, 1024] as fp16
                    # (values bounded ~4e3, fp16-safe), transpose each
                    # 128-col chunk to q-major via the DMA xbar (rows
                    # padded to 80 = 5 xbar tiles; cols 65-79 are junk
                    # and never read), then normalize by column 64.
                    pv_h = pvhp.tile([80, QHW], fp16, tag="pvh")
                    o_t = otp.tile([128, NBLK, 80], fp16, tag="ot")
                    o_f = of32p.tile([128, NBLK, D], f32, tag="of")

                    def evac_job(pv=pv, pv_h=pv_h):
                        nc.vector.tensor_copy(out=pv_h[0:D + 1, :], in_=pv)
                    pending.append(evac_job)

                    for blk in range(NBLK):
                        def tr_job(blk=blk, pv_h=pv_h, o_t=o_t):
                            nc.sync.dma_start_transpose(
                                out=o_t[:, blk, :],
                                in_=pv_h[:, blk * 128:(blk + 1) * 128])
                        pending.append(tr_job)

                    for blk in range(NBLK):
                        def norm_job(blk=blk, o_t=o_t, o_f=o_f):
                            rz = rzp.tile([128, 1], f32, tag="rz")
                            nc.vector.reciprocal(
                                out=rz, in_=o_t[:, blk, D:D + 1])
                            nc.vector.tensor_scalar_mul(
                                out=o_f[:, blk, :], in0=o_t[:, blk, 0:D],
                                scalar1=rz)
                        pending.append(norm_job)

                    def store_job(b=b, q0=q0, o_f=o_f):
                        nc.sync.dma_start(
                            out=o_d.ap()[b, q0:q0 + QHW, :].rearrange(
                                "(t p) d -> p t d", p=128),
                            in_=o_f)
                    pending.append(store_job)

                while slot < len(next_jobs):
                    next_jobs[slot]()
                    slot += 1

            for _ in range(12):
                warmer()
            jobs0 = prep_load(0)
            for job in jobs0:
                job()
            for b in range(B_SH):
                nxt = prep_load(b + 1) if b + 1 < B_SH else []
                main(b, nxt)
            for job in pending:
                job()

    nc.finalize()
    return nc


def _get_nc():
    if "nc" not in _cached:
        _cached["nc"] = _build()
    return _cached["nc"]


def kernel(query, key, value):
    from concourse.bass_utils import run_bass_kernel_spmd

    nc = _get_nc()
    query = np.ascontiguousarray(query, dtype=np.float32)
    key = np.ascontiguousarray(key, dtype=np.float32)
    value = np.ascontiguousarray(value, dtype=np.float32)

    in_maps = []
    for c in range(NCORES):
        sl = slice(c * B_SH, (c + 1) * B_SH)
        in_maps.append({
            "query": query[sl], "key": key[sl], "value": value[sl]})

    res = run_bass_kernel_spmd(nc, in_maps, core_ids=list(range(NCORES)))
    out = np.concatenate([r["out"] for r in res.results], axis=0)
    return out


# revision 19
# speedup vs baseline: 1.1088x; 1.1088x over previous
"""Dense dot-product attention on 8 Trainium2 NeuronCores.

Problem: query/key/value [32, 2048, 64] fp32 -> softmax(Q K^T / 8) V.
Sharding: batch dim split 4-per-core across 8 cores (data parallel, no
collectives). Each core computes full attention for its 4 batches.

Design: the kernel is ScalarE-bound (exp of all B*L*L scores at 1
elem/cycle/lane, ~147us/core floor at N=1024 activations); everything
else is arranged so the PE + DVE + DMA stay below that floor and the
exp stream never stalls.

  - All matmuls fp16, and every matmul is a FULL 128x128-mode op: the
    HAM clock monitor ignores row-tiled matmuls, and a 64-row-mode
    kernel is stuck at 1.2 GHz. The S matmul gets a full C=128
    contraction by zero-padding: khT3[:, t, :] holds K^T tile t in one
    partition half (even tiles low, odd tiles high) and ZEROS in the
    other, assembled by DMA-xbar pair transposes + per-pair shuffles.
  - Q^T is built with PE transposes (identity matmul) woven into the
    previous batch's stream, copied to qhT[0:64] and duplicated into
    partitions 64-127 so the rhs can span the full contraction.
  - Per k-tile pipeline: S (2 matmuls, N=512) -> exp (ScalarE,
    N=1024, PSUM source) -> PV for the *previous* tile, so the PE
    streams under the current exp. Score tiles rotate through 2 PSUM
    buffers (distance 2 is enough: the exp that last read a buffer
    finishes one slot before the S write reissues it).
  - PV is V-stationary: lhsT = [V | ones] [128, 65], rhs = the exp'd
    scores (N=512) accumulating out^T [65, q] over the 16 k-tiles;
    row 64 is the softmax denominator for free.
  - Out: evacuate [65, 1024] to SBUF, PE-transpose 128-col chunks,
    reciprocal of the denominator column + per-block scale on DVE.
"""

import numpy as np

B, L, D = 32, 2048, 64
NCORES = 8
B_SH = B // NCORES          # 4 batches per core
LT = L // 128               # 16 k-tiles of 128
NQH = 2                     # q processed in halves of 1024
QHW = L // NQH              # 1024
NBLK = QHW // 128           # 8 q-blocks of 128 per qh
SCALE = 1.0 / np.sqrt(np.float32(D))  # 0.125

_cached = {}


def _build():
    import concourse.bacc as bacc
    import concourse.tile as tile
    from concourse import mybir
    from concourse.masks import make_identity

    f32 = mybir.dt.float32
    fp16 = mybir.dt.float16
    Exp = mybir.ActivationFunctionType.Exp

    nc = bacc.Bacc("TRN2", target_bir_lowering=False, debug=False)

    q_d = nc.dram_tensor("query", [B_SH, L, D], f32, kind="ExternalInput")
    k_d = nc.dram_tensor("key", [B_SH, L, D], f32, kind="ExternalInput")
    v_d = nc.dram_tensor("value", [B_SH, L, D], f32, kind="ExternalInput")
    o_d = nc.dram_tensor("out", [B_SH, L, D], f32, kind="ExternalOutput")

    with tile.TileContext(nc) as tc:
        with (
            tc.tile_pool(name="consts", bufs=1) as consts,
            tc.tile_pool(name="nat", bufs=2) as nat,
            tc.tile_pool(name="nath", bufs=2) as nath,
            tc.tile_pool(name="vst", bufs=2) as vst,
            tc.tile_pool(name="qkt", bufs=2) as qkt,
            tc.tile_pool(name="vr", bufs=2) as vrp,
            tc.tile_pool(name="er", bufs=2) as erp,
            tc.tile_pool(name="pvsb", bufs=2) as pvsbp,
            tc.tile_pool(name="oall", bufs=2) as oallp,
            tc.tile_pool(name="rz", bufs=8) as rzp,
            tc.tile_pool(name="sps", bufs=2, space="PSUM") as sps,
            tc.tile_pool(name="pvps", bufs=1, space="PSUM") as pvps,
            tc.tile_pool(name="trps", bufs=2, space="PSUM") as trps,
        ):
            # warmers + ACT table load first: they run during the
            # initial DMA latency and before the gpsimd identity build
            # (~6us first-custom-op IRAM load).
            wsrc = consts.tile([128, 512], fp16)
            nc.vector.memset(wsrc, 1.0)
            dummy = consts.tile([128, 1], f32)
            nc.vector.memset(dummy, 0.0)
            nc.scalar.activation(out=dummy, in_=dummy, func=Exp, scale=1.0)
            ident = consts.tile([128, 128], f32)
            make_identity(nc, ident)
            identh = consts.tile([128, 128], fp16)
            nc.vector.tensor_copy(out=identh, in_=ident)

            warm_ps = pvps.tile([D + 1, QHW], f32, tag="pv")

            def warmer(n=512):
                nc.tensor.matmul(warm_ps[0:64, 0:n], wsrc[:, 0:64],
                                 wsrc[:, 0:n], start=True, stop=True,
                                 skip_group_check=True)

            # per-batch persistent tiles
            tiles = {}  # b -> (qhT, khT3, vr)

            def prep_load(b):
                """Loads + casts + transposes for batch b, as a list of
                thunks. Placement (weave order) controls both issue
                order and -- because Tile hazards are object-granular
                -- the dependency frontier each consumer sees.
                """
                q_nat = nat.tile([128, LT, D], f32, tag="qnat")
                k_nat = nat.tile([128, LT, D], f32, tag="knat")
                qh_nat = nath.tile([128, LT, D], fp16, tag="qh_nat")
                kh_nat = nath.tile([128, LT, D], fp16, tag="kh_nat")
                qhT = qkt.tile([128, L], fp16, tag="qhT")
                khT3 = qkt.tile([128, LT, 128], fp16, tag="khT3")
                kstage = qkt.tile([128, LT // 2, 128], fp16, tag="kstage")
                v_stage = vst.tile([128, LT, D], f32, tag="vstage")
                vr = vrp.tile([128, LT, D + 1], fp16, tag="vr")

                q_r = q_d.ap()[b].rearrange("(t p) d -> p t d", p=128)
                k_r = k_d.ap()[b].rearrange("(t p) d -> p t d", p=128)
                khT3_v = khT3.rearrange("p (t e) c -> p t e c", e=2)

                jobs = []

                def j(fn):
                    jobs.append(fn)

                def k_chunk(c0, c1):
                    j(lambda: nc.sync.dma_start(
                        out=k_nat[:, c0:c1, :], in_=k_r[:, c0:c1, :]))
                    j(lambda: nc.vector.tensor_copy(
                        out=kh_nat[:, c0:c1, :], in_=k_nat[:, c0:c1, :]))

                def q_chunk(c0, c1):
                    j(lambda: nc.sync.dma_start(
                        out=q_nat[:, c0:c1, :], in_=q_r[:, c0:c1, :]))
                    j(lambda: nc.vector.tensor_copy(
                        out=qh_nat[:, c0:c1, :], in_=q_nat[:, c0:c1, :]))

                # K tile-pair transpose via the DMA xbar (~1.2us of
                # Sync-engine time each -- only 8/batch, the Q path
                # stays off Sync), then shuffle the halves into
                # zero-padded per-tile blocks.
                def ktr(p):
                    def f():
                        nc.sync.dma_start_transpose(
                            out=kstage[:, p, :],
                            in_=kh_nat[:, 2 * p:2 * p + 2, :])
                        nc.sync.dma_start(
                            out=khT3_v[0:64, p, 0, :], in_=kstage[0:64, p, :])
                        nc.sync.dma_start(
                            out=khT3_v[64:128, p, 1, :],
                            in_=kstage[64:128, p, :])
                    j(f)

                # Q tile transpose on the PE + copy into qhT + per-pair
                # duplication into the upper partition half.
                def qtr(t):
                    def f():
                        tp = trps.tile([64, 128], fp16, tag="tr")
                        nc.tensor.transpose(tp, qh_nat[:, t, :], identh)
                        nc.vector.tensor_copy(
                            out=qhT[0:64, t * 128:(t + 1) * 128], in_=tp)
                    j(f)

                def qdup(p):
                    def f():
                        nc.sync.dma_start(
                            out=qhT[64:128, 2 * p * 128:(2 * p + 2) * 128],
                            in_=qhT[0:64, 2 * p * 128:(2 * p + 2) * 128])
                    j(f)

                if b < 2:
                    # zero halves of khT3 are only written by these
                    # memsets; buffers rotate 2-deep so batches 2,3
                    # reuse batch 0,1's zeros.
                    j(lambda: nc.vector.memset(khT3_v[64:128, :, 0, :], 0.0))
                    j(lambda: nc.vector.memset(khT3_v[0:64, :, 1, :], 0.0))
                # priority order: everything batch b's FIRST qh touches
                # comes first (qh0 S matmuls read qhT[:, 0:1024] =
                # Q tiles 0-7 + dups 0-3, and khT3); under
                # object-granular hazards a read sees only writes
                # issued before it, so qh1's Q tiles must also be
                # issued (woven) before qh1's S matmuls -- the weave
                # guarantees that.
                k_chunk(0, 2)
                ktr(0)
                q_chunk(0, 8)
                for t in range(8):
                    qtr(t)
                for p in range(4):
                    qdup(p)
                k_chunk(2, 8)
                ktr(1)
                # --- batch-0 inline prefix ends here ---
                ktr(2)
                ktr(3)
                k_chunk(8, LT)
                for p in range(4, 8):
                    ktr(p)
                j(lambda: nc.sync.dma_start(
                    out=v_stage,
                    in_=v_d.ap()[b].rearrange("(t p) d -> p t d", p=128)))
                q_chunk(8, LT)
                for t in range(8, LT):
                    qtr(t)
                for p in range(4, 8):
                    qdup(p)
                j(lambda: nc.vector.tensor_copy(out=vr[:, :, 0:D], in_=v_stage))
                j(lambda: nc.vector.memset(vr[:, :, D:D + 1], 1.0))

                tiles[b] = (qhT, khT3, vr)
                return jobs

            pending = []   # deferred qh-finish jobs woven into later slots

            def main(b, next_jobs, wps=2):
                qhT, khT3, vr = tiles.pop(b)
                slot = 0

                def weave(n):
                    nonlocal slot
                    if pending:
                        pending.pop(0)()
                    for _ in range(n):
                        if slot < len(next_jobs):
                            next_jobs[slot]()
                            slot += 1

                def s_mms(s, t, q0):
                    w = khT3[:, t, :]
                    for jj in range(QHW // 512):
                        nc.tensor.matmul(
                            s[:, jj * 512:(jj + 1) * 512],
                            w, qhT[:, q0 + jj * 512:q0 + (jj + 1) * 512],
                            start=True, stop=True)

                def pv_mms(pv, e_of_t):
                    for t, e_chunk in e_of_t:
                        for jj in range(QHW // 512):
                            nc.tensor.matmul(
                                pv[:, jj * 512:(jj + 1) * 512],
                                vr[:, t, :],
                                e_chunk[:, jj * 512:(jj + 1) * 512],
                                start=(t == 0), stop=(t == LT - 1))

                for qh in range(NQH):
                    q0 = qh * QHW
                    pv = pvps.tile([D + 1, QHW], f32, tag="pv")
                    prev_pv = None

                    for t in range(LT):
                        weave(wps)
                        s = sps.tile([128, QHW], f32, tag="s")
                        s_mms(s, t, q0)
                        e = erp.tile([128, QHW], fp16, tag="e")
                        nc.scalar.activation(
                            out=e, in_=s, func=Exp, scale=float(SCALE))
                        if prev_pv is not None:
                            pv_mms(*prev_pv)
                        prev_pv = (pv, [(t, e)])

                    # last tile's PV + evacuation run inline so pv
                    # (bufs=1) is free before the next qh needs it
                    pv_mms(*prev_pv)
                    pv_sb = pvsbp.tile([D + 1, QHW], f32, tag="pvsb")
                    nc.vector.tensor_copy(out=pv_sb, in_=pv)

                    o_all = oallp.tile([128, NBLK, D], f32, tag="oall")
                    for qt in range(NBLK):
                        def out_job(qt=qt, pv_sb=pv_sb, o_all=o_all):
                            ot = trps.tile([128, D + 1], f32, tag="tr")
                            nc.tensor.transpose(
                                ot, pv_sb[:, qt * 128:(qt + 1) * 128],
                                ident[0:D + 1, 0:D + 1])
                            rz = rzp.tile([128, 1], f32, tag="rz")
                            nc.vector.reciprocal(out=rz, in_=ot[:, D:D + 1])
                            nc.vector.tensor_scalar_mul(
                                out=o_all[:, qt, :], in0=ot[:, 0:D],
                                scalar1=rz)
                        pending.append(out_job)

                    def store_job(b=b, q0=q0, o_all=o_all):
                        nc.sync.dma_start(
                            out=o_d.ap()[b, q0:q0 + QHW, :].rearrange(
                                "(t p) d -> p t d", p=128),
                            in_=o_all)
                    pending.append(store_job)

                while slot < len(next_jobs):
                    next_jobs[slot]()
                    slot += 1

            for _ in range(12):
                warmer()
            jobs0 = prep_load(0)
            # fast start: inline the prefix main(0)'s qh0 needs
            # (K pairs 0-1, Q tiles 0-7 + dups 0-3); weave the rest.
            n_inline = len(jobs0)  # bisect: all batch-0 prep inline
            for job in jobs0[:n_inline]:
                job()
            for b in range(B_SH):
                nxt = prep_load(b + 1) if b + 1 < B_SH else []
                if b == 0:
                    main(b, jobs0[n_inline:] + nxt, wps=3)
                else:
                    main(b, nxt)
            for job in pending:
                job()

    nc.finalize()
    return nc


def _get_nc():
    if "nc" not in _cached:
        _cached["nc"] = _build()
    return _cached["nc"]


def kernel(query, key, value):
    from concourse.bass_utils import run_bass_kernel_spmd

    nc = _get_nc()
    query = np.ascontiguousarray(query, dtype=np.float32)
    key = np.ascontiguousarray(key, dtype=np.float32)
    value = np.ascontiguousarray(value, dtype=np.float32)

    in_maps = []
    for c in range(NCORES):
        sl = slice(c * B_SH, (c + 1) * B_SH)
        in_maps.append({
            "query": query[sl], "key": key[sl], "value": value[sl]})

    res = run_bass_kernel_spmd(nc, in_maps, core_ids=list(range(NCORES)))
    out = np.concatenate([r["out"] for r in res.results], axis=0)
    return out


# revision 20
# speedup vs baseline: 1.5008x; 1.3534x over previous
"""Dense dot-product attention on 8 Trainium2 NeuronCores.

Problem: query/key/value [32, 2048, 64] fp32 -> softmax(Q K^T / 8) V.
Sharding: batch dim split 4-per-core across 8 cores (data parallel, no
collectives). Each core computes full attention for its 4 batches.

All matmuls run in fp16: 1 cycle/column on the PE. Crucially, every
matmul is a FULL 128x128-mode op: the PE's HAM activity monitor does
not count row-tiled (64-row-mode) matmuls as activity, so a kernel
whose S matmuls use a 64-deep contraction oscillates between 1.2 and
2.4 GHz. The S matmul gets a full C=128 contraction by zero-padding:
khT3[:, t, :] holds K^T tile t in partitions 0-63 (even t) or 64-127
(odd t) and ZEROS in the other half, so lhsT spans all 128 partitions
while computing exactly the same scores.

Per-batch dataflow:
  1. DMA Q,K natural [2048,64]; DVE-cast to fp16; PE-transpose 128-row
     tiles; Q^T -> [64,2048] in SBUF duplicated into both partition
     halves (the rhs also spans the full contraction); K^T -> the
     zero-padded khT3 blocks.
  2. S^T[k,q] = khT3[:,t].T @ qhT, two k-tiles per kp slot, into fp32
     PSUM [128k, 1024q] blocks.
  3. exp on ScalarE straight out of PSUM (scale=1/8 folded in), fp16
     out. No max-subtraction: scores ~ N(0,1), exp cannot overflow.
  4. P@V via fp16 matmul with lhsT = [V | ones] [128k, 65]: accumulates
     out^T [65, q] in fp32 PSUM over the 16 k-tiles; row 64 = softmax
     denominator.
  5. PE-transpose out^T chunks -> [128q, 65], DVE reciprocal of col 64,
     row-scale cols 0..63, DMA out.

The next batch's input transposes are interleaved into the current
batch's matmul stream so the PE and ScalarE never drain between batches.
"""

import numpy as np

B, L, D = 32, 2048, 64
NCORES = 8
B_SH = B // NCORES          # 4 batches per core
LT = L // 128               # 16 k/l tiles of 128
NQH = 2                     # q processed in halves of 1024
QHW = L // NQH              # 1024
SCALE = 1.0 / np.sqrt(np.float32(D))  # 0.125

_cached = {}


def _build():
    import concourse.bacc as bacc
    import concourse.tile as tile
    from concourse import mybir
    from concourse.masks import make_identity

    f32 = mybir.dt.float32
    fp16 = mybir.dt.float16
    Exp = mybir.ActivationFunctionType.Exp

    nc = bacc.Bacc("TRN2", target_bir_lowering=False, debug=False)

    q_d = nc.dram_tensor("query", [B_SH, L, D], f32, kind="ExternalInput")
    k_d = nc.dram_tensor("key", [B_SH, L, D], f32, kind="ExternalInput")
    v_d = nc.dram_tensor("value", [B_SH, L, D], f32, kind="ExternalInput")
    o_d = nc.dram_tensor("out", [B_SH, L, D], f32, kind="ExternalOutput")

    with tile.TileContext(nc) as tc:
        with (
            tc.tile_pool(name="consts", bufs=1) as consts,
            tc.tile_pool(name="nat", bufs=2) as nat,
            tc.tile_pool(name="nath", bufs=2) as nath,
            tc.tile_pool(name="vst", bufs=2) as vst,
            tc.tile_pool(name="qkt", bufs=2) as qkt,
            tc.tile_pool(name="vr", bufs=2) as vrp,
            tc.tile_pool(name="er", bufs=4) as erp,
            tc.tile_pool(name="pvsb", bufs=3) as pvsb,
            tc.tile_pool(name="oall", bufs=3) as oallp,
            tc.tile_pool(name="rz", bufs=8) as rzp,
            tc.tile_pool(name="sps", bufs=2, space="PSUM") as sps,
            tc.tile_pool(name="pvps", bufs=1, space="PSUM") as pvps,
            tc.tile_pool(name="trps", bufs=2, space="PSUM") as trps,
        ):
            # wsrc/dummy first: the warm-up burst and ACT table load
            # must not queue behind the gpsimd identity build (~6us
            # first-custom-op IRAM load)
            wsrc = consts.tile([128, 512], fp16)
            nc.vector.memset(wsrc, 1.0)
            dummy = consts.tile([128, 1], f32)
            nc.vector.memset(dummy, 0.0)
            nc.scalar.activation(out=dummy, in_=dummy, func=Exp, scale=1.0)
            ident = consts.tile([128, 128], f32)
            make_identity(nc, ident)
            identh = consts.tile([128, 128], fp16)
            nc.vector.tensor_copy(out=identh, in_=ident)

            def warmer(n=512):
                wt = trps.tile([64, 512], f32, tag="tr")
                nc.tensor.matmul(wt[:, 0:n], wsrc[:, 0:64], wsrc[:, 0:n],
                                 start=True, stop=True, skip_group_check=True)

            # per-batch persistent tiles
            qkT = {}   # b -> (qhT [128,2048] dup-halves, khT3 [128,16,128] zero-padded)
            v_r = {}   # b -> [128, 16, 65] fp16  (col 64 = 1.0)

            def prep_load(b):
                """DMA loads + fp16 casts + transpose jobs for batch b."""
                q_nat = nat.tile([128, LT, D], f32, tag="qnat")
                k_nat = nat.tile([128, LT, D], f32, tag="knat")
                q_r = q_d.ap()[b].rearrange("(t p) d -> p t d", p=128)
                k_r = k_d.ap()[b].rearrange("(t p) d -> p t d", p=128)
                # split loads so the first tiles (and their casts) land
                # early: main(qh=0, kp=0) needs Q tiles 0-7, K tiles 0-1
                nc.sync.dma_start(out=k_nat[:, 0:2, :], in_=k_r[:, 0:2, :])
                nc.sync.dma_start(out=q_nat[:, 0:8, :], in_=q_r[:, 0:8, :])
                nc.sync.dma_start(out=k_nat[:, 2:LT, :], in_=k_r[:, 2:LT, :])
                nc.sync.dma_start(out=q_nat[:, 8:LT, :], in_=q_r[:, 8:LT, :])

                qh_nat = nath.tile([128, LT, D], fp16, tag="qh_nat")
                kh_nat = nath.tile([128, LT, D], fp16, tag="kh_nat")
                nc.vector.tensor_copy(out=kh_nat[:, 0:2, :], in_=k_nat[:, 0:2, :])
                nc.vector.tensor_copy(out=qh_nat[:, 0:8, :], in_=q_nat[:, 0:8, :])
                nc.vector.tensor_copy(out=kh_nat[:, 2:LT, :], in_=k_nat[:, 2:LT, :])
                nc.vector.tensor_copy(out=qh_nat[:, 8:LT, :], in_=q_nat[:, 8:LT, :])

                qhT = qkt.tile([128, L], fp16, tag="qhT")
                khT3 = qkt.tile([128, LT, 128], fp16, tag="khT3")
                if b < 2:
                    # zero halves of khT3: even tiles live in partitions
                    # 0-63, odd in 64-127; the complement half must be 0
                    # so the full-C matmul adds nothing. Only these
                    # memsets ever write the complements; buffers rotate
                    # 2-deep so batches 2,3 reuse batch 0,1's zeros.
                    # Issued immediately (prep_load runs before the
                    # previous batch's main), so they execute early.
                    khT3_v = khT3.rearrange("p (t e) c -> p t e c", e=2)
                    nc.vector.memset(khT3_v[64:128, :, 0, :], 0.0)
                    nc.vector.memset(khT3_v[0:64, :, 1, :], 0.0)

                v_stage = vst.tile([128, LT, D], f32, tag="vstage")
                nc.sync.dma_start(
                    out=v_stage, in_=v_d.ap()[b].rearrange("(t p) d -> p t d", p=128))
                vr = vrp.tile([128, LT, D + 1], fp16, tag="vr")
                nc.vector.tensor_copy(out=vr[:, :, 0:D], in_=v_stage)
                nc.vector.memset(vr[:, :, D:D + 1], 1.0)

                qkT[b] = (qhT, khT3)
                v_r[b] = vr

                jobs = []
                for lt in range(LT):
                    def q_tr_job(lt=lt):
                        tp = trps.tile([64, 128], fp16, tag="tr")
                        nc.tensor.transpose(tp, qh_nat[:, lt, :], identh)
                        nc.vector.tensor_copy(
                            out=qhT[0:64, lt * 128:(lt + 1) * 128], in_=tp)

                    def k_tr_job(lt=lt):
                        tp = trps.tile([64, 128], fp16, tag="tr")
                        nc.tensor.transpose(tp, kh_nat[:, lt, :], identh)
                        h = slice(0, 64) if lt % 2 == 0 else slice(64, 128)
                        nc.vector.tensor_copy(out=khT3[h, lt, :], in_=tp)

                    jobs.append(q_tr_job)
                    jobs.append(k_tr_job)

                def qdup_job():
                    nc.sync.dma_start(out=qhT[64:128, :], in_=qhT[0:64, :])

                jobs.append(qdup_job)
                return jobs

            pending = []   # deferred small jobs woven into the MM stream

            def main(b, next_jobs, weave=2):
                qhT, khT3 = qkT.pop(b)
                vr = v_r.pop(b)
                slot = 0
                for qh in range(NQH):
                    q0 = qh * QHW
                    pv = pvps.tile([D + 1, QHW], f32, tag="pv")

                    for kp in range(LT // 2):      # pairs of k-tiles
                        ka, kb = 2 * kp, 2 * kp + 1
                        # interleave deferred out-work + next batch's prep
                        if pending:
                            pending.pop(0)()
                        for _ in range(weave):
                            if slot < len(next_jobs):
                                next_jobs[slot]()
                                slot += 1
                        s_a = sps.tile([128, QHW], f32, tag="s")
                        s_b = sps.tile([128, QHW], f32, tag="s")
                        # full-C (128) matmuls against the zero-padded
                        # K^T blocks; rhs spans both duplicated halves
                        for s_ps, kt in ((s_a, ka), (s_b, kb)):
                            for j in range(QHW // 512):
                                js = slice(j * 512, (j + 1) * 512)
                                qs = slice(q0 + j * 512, q0 + (j + 1) * 512)
                                nc.tensor.matmul(
                                    s_ps[:, js], khT3[:, kt, :], qhT[:, qs],
                                    start=True, stop=True)
                        for kt, s_ps in ((ka, s_a), (kb, s_b)):
                            e_r = erp.tile([128, QHW], fp16, tag="e")
                            nc.scalar.activation(out=e_r, in_=s_ps, func=Exp,
                                                 scale=float(SCALE))
                            for j in range(QHW // 512):
                                js = slice(j * 512, (j + 1) * 512)
                                nc.tensor.matmul(
                                    pv[:, js], vr[:, kt, :], e_r[:, js],
                                    start=(kt == 0), stop=(kt == LT - 1))

                    # defer psum evacuation + out-transpose + normalize:
                    # woven into subsequent pair-slots so the in-order PE
                    # stream never blocks on this at the qh boundary
                    pv_sb = pvsb.tile([D + 1, QHW], f32, tag="pvsb")
                    o_all = oallp.tile([128, QHW // 128, D], f32, tag="oall")

                    def evac_job(pv=pv, pv_sb=pv_sb):
                        nc.vector.tensor_copy(out=pv_sb, in_=pv)
                    pending.append(evac_job)

                    for qt in range(QHW // 128):
                        def out_job(qt=qt, pv_sb=pv_sb, o_all=o_all):
                            ot = trps.tile([128, D + 1], f32, tag="tr")
                            nc.tensor.transpose(
                                ot, pv_sb[:, qt * 128:(qt + 1) * 128],
                                ident[0:D + 1, 0:D + 1])
                            rz = rzp.tile([128, 1], f32, tag="rz")
                            nc.vector.reciprocal(out=rz, in_=ot[:, D:D + 1])
                            nc.vector.tensor_scalar_mul(
                                out=o_all[:, qt, :], in0=ot[:, 0:D],
                                scalar1=rz)
                        pending.append(out_job)

                    def store_job(b=b, q0=q0, o_all=o_all):
                        nc.sync.dma_start(
                            out=o_d.ap()[b, q0:q0 + QHW, :].rearrange(
                                "(t p) d -> p t d", p=128),
                            in_=o_all)
                    pending.append(store_job)
                while slot < len(next_jobs):
                    next_jobs[slot]()
                    slot += 1

            for _ in range(12):
                warmer()
            jobs0 = prep_load(0)
            tr0 = jobs0[:2 * LT]
            # fast start: inline only what main(0) qh=0 kp=0 needs --
            # Q tiles 0-7 (+ region dup), K tiles 0-1
            for job in [tr0[1], tr0[3]] + [tr0[2 * l] for l in range(8)]:
                job()
            qhT0, khT3_0 = qkT[0]
            nc.sync.dma_start(out=qhT0[64:128, 0:QHW], in_=qhT0[0:64, 0:QHW])

            def region_qdup(lo, hi):
                def job():
                    nc.sync.dma_start(out=qhT0[64:128, lo:hi], in_=qhT0[0:64, lo:hi])
                return job

            # remaining K tiles in consumption order, then Q tiles 8-15
            # for qh=1 plus their upper-half dup
            rest = []
            for l in range(2, LT):
                rest.append(tr0[2 * l + 1])
            rest += [tr0[2 * l] for l in range(8, LT)]
            rest.append(region_qdup(QHW, L))
            for b in range(B_SH):
                nxt = prep_load(b + 1) if b + 1 < B_SH else []
                if b == 0:
                    main(b, rest + nxt, weave=5)
                else:
                    main(b, nxt)
            for job in pending:
                job()

    nc.finalize()
    return nc


def _get_nc():
    if "nc" not in _cached:
        _cached["nc"] = _build()
    return _cached["nc"]


def kernel(query, key, value):
    from concourse.bass_utils import run_bass_kernel_spmd

    nc = _get_nc()
    query = np.ascontiguousarray(query, dtype=np.float32)
    key = np.ascontiguousarray(key, dtype=np.float32)
    value = np.ascontiguousarray(value, dtype=np.float32)

    in_maps = []
    for c in range(NCORES):
        sl = slice(c * B_SH, (c + 1) * B_SH)
        in_maps.append({
            "query": query[sl], "key": key[sl], "value": value[sl]})

    res = run_bass_kernel_spmd(nc, in_maps, core_ids=list(range(NCORES)))
    out = np.concatenate([r["out"] for r in res.results], axis=0)
    return out


# revision 21
# speedup vs baseline: 1.5437x; 1.0287x over previous
"""Dense dot-product attention on 8 Trainium2 NeuronCores.

Problem: query/key/value [32, 2048, 64] fp32 -> softmax(Q K^T / 8) V.
Sharding: batch dim split 4-per-core across 8 cores (data parallel, no
collectives). Each core computes full attention for its 4 batches.

All matmuls run in fp16: 1 cycle/column on the PE. Crucially, every
matmul is a FULL 128x128-mode op: the PE's HAM activity monitor does
not count row-tiled (64-row-mode) matmuls as activity, so a kernel
whose S matmuls use a 64-deep contraction oscillates between 1.2 and
2.4 GHz. The S matmul gets a full C=128 contraction by zero-padding:
khT3[:, t, :] holds K^T tile t in partitions 0-63 (even t) or 64-127
(odd t) and ZEROS in the other half, so lhsT spans all 128 partitions
while computing exactly the same scores.

Per-batch dataflow:
  1. DMA Q,K natural [2048,64]; DVE-cast to fp16; PE-transpose 128-row
     tiles; Q^T -> [64,2048] in SBUF duplicated into both partition
     halves (the rhs also spans the full contraction); K^T -> the
     zero-padded khT3 blocks.
  2. S^T[k,q] = khT3[:,t].T @ qhT, two k-tiles per kp slot, into fp32
     PSUM [128k, 1024q] blocks.
  3. exp on ScalarE straight out of PSUM (scale=1/8 folded in), fp16
     out. No max-subtraction: scores ~ N(0,1), exp cannot overflow.
  4. P@V via fp16 matmul with lhsT = [V | ones] [128k, 65]: accumulates
     out^T [65, q] in fp32 PSUM over the 16 k-tiles; row 64 = softmax
     denominator.
  5. PE-transpose out^T chunks -> [128q, 65], DVE reciprocal of col 64,
     row-scale cols 0..63, DMA out.

The next batch's input transposes are interleaved into the current
batch's matmul stream so the PE and ScalarE never drain between batches.
"""

import numpy as np

B, L, D = 32, 2048, 64
NCORES = 8
B_SH = B // NCORES          # 4 batches per core
LT = L // 128               # 16 k/l tiles of 128
NQH = 2                     # q processed in halves of 1024
QHW = L // NQH              # 1024
SCALE = 1.0 / np.sqrt(np.float32(D))  # 0.125

_cached = {}


def _build():
    import concourse.bacc as bacc
    import concourse.tile as tile
    from concourse import mybir
    from concourse.masks import make_identity

    f32 = mybir.dt.float32
    fp16 = mybir.dt.float16
    Exp = mybir.ActivationFunctionType.Exp

    nc = bacc.Bacc("TRN2", target_bir_lowering=False, debug=False)

    q_d = nc.dram_tensor("query", [B_SH, L, D], f32, kind="ExternalInput")
    k_d = nc.dram_tensor("key", [B_SH, L, D], f32, kind="ExternalInput")
    v_d = nc.dram_tensor("value", [B_SH, L, D], f32, kind="ExternalInput")
    o_d = nc.dram_tensor("out", [B_SH, L, D], f32, kind="ExternalOutput")

    with tile.TileContext(nc) as tc:
        with (
            tc.tile_pool(name="consts", bufs=1) as consts,
            tc.tile_pool(name="nat", bufs=2) as nat,
            tc.tile_pool(name="nath", bufs=2) as nath,
            tc.tile_pool(name="vst", bufs=2) as vst,
            tc.tile_pool(name="qkt", bufs=2) as qkt,
            tc.tile_pool(name="vr", bufs=2) as vrp,
            tc.tile_pool(name="er", bufs=4) as erp,
            tc.tile_pool(name="pvsb", bufs=3) as pvsb,
            tc.tile_pool(name="oall", bufs=3) as oallp,
            tc.tile_pool(name="rz", bufs=8) as rzp,
            tc.tile_pool(name="sps", bufs=2, space="PSUM") as sps,
            tc.tile_pool(name="pvps", bufs=1, space="PSUM") as pvps,
            tc.tile_pool(name="trps", bufs=2, space="PSUM") as trps,
        ):
            # wsrc/dummy first: the warm-up burst and ACT table load
            # must not queue behind the gpsimd identity build (~6us
            # first-custom-op IRAM load)
            wsrc = consts.tile([128, 512], fp16)
            nc.vector.memset(wsrc, 1.0)
            dummy = consts.tile([128, 1], f32)
            nc.vector.memset(dummy, 0.0)
            nc.scalar.activation(out=dummy, in_=dummy, func=Exp, scale=1.0)
            ident = consts.tile([128, 128], f32)
            make_identity(nc, ident)
            identh = consts.tile([128, 128], fp16)
            nc.vector.tensor_copy(out=identh, in_=ident)

            def warmer(n=512):
                wt = trps.tile([64, 512], f32, tag="tr")
                nc.tensor.matmul(wt[:, 0:n], wsrc[:, 0:64], wsrc[:, 0:n],
                                 start=True, stop=True, skip_group_check=True)

            # per-batch persistent tiles
            qkT = {}   # b -> (qhT [128,2048] dup-halves, khT3 [128,16,128] zero-padded)
            v_r = {}   # b -> [128, 16, 65] fp16  (col 64 = 1.0)

            def prep_load(b):
                """DMA loads + fp16 casts + transpose jobs for batch b."""
                q_nat = nat.tile([128, LT, D], f32, tag="qnat")
                k_nat = nat.tile([128, LT, D], f32, tag="knat")
                q_r = q_d.ap()[b].rearrange("(t p) d -> p t d", p=128)
                k_r = k_d.ap()[b].rearrange("(t p) d -> p t d", p=128)
                # split loads so the first tiles (and their casts) land
                # early: main(qh=0, kp=0) needs Q tiles 0-7, K tiles 0-1
                nc.sync.dma_start(out=k_nat[:, 0:2, :], in_=k_r[:, 0:2, :])
                nc.sync.dma_start(out=k_nat[:, 2:6, :], in_=k_r[:, 2:6, :])
                nc.sync.dma_start(out=q_nat[:, 0:8, :], in_=q_r[:, 0:8, :])
                nc.sync.dma_start(out=k_nat[:, 6:LT, :], in_=k_r[:, 6:LT, :])
                nc.sync.dma_start(out=q_nat[:, 8:LT, :], in_=q_r[:, 8:LT, :])

                qh_nat = nath.tile([128, LT, D], fp16, tag="qh_nat")
                kh_nat = nath.tile([128, LT, D], fp16, tag="kh_nat")
                nc.vector.tensor_copy(out=kh_nat[:, 0:2, :], in_=k_nat[:, 0:2, :])
                nc.vector.tensor_copy(out=kh_nat[:, 2:6, :], in_=k_nat[:, 2:6, :])
                nc.vector.tensor_copy(out=qh_nat[:, 0:8, :], in_=q_nat[:, 0:8, :])
                nc.vector.tensor_copy(out=kh_nat[:, 6:LT, :], in_=k_nat[:, 6:LT, :])
                nc.vector.tensor_copy(out=qh_nat[:, 8:LT, :], in_=q_nat[:, 8:LT, :])

                qhT = qkt.tile([128, L], fp16, tag="qhT")
                khT3 = qkt.tile([128, LT, 128], fp16, tag="khT3")
                if b < 2:
                    # zero halves of khT3: even tiles live in partitions
                    # 0-63, odd in 64-127; the complement half must be 0
                    # so the full-C matmul adds nothing. Only these
                    # memsets ever write the complements; buffers rotate
                    # 2-deep so batches 2,3 reuse batch 0,1's zeros.
                    # Issued immediately (prep_load runs before the
                    # previous batch's main), so they execute early.
                    khT3_v = khT3.rearrange("p (t e) c -> p t e c", e=2)
                    nc.vector.memset(khT3_v[64:128, :, 0, :], 0.0)
                    nc.vector.memset(khT3_v[0:64, :, 1, :], 0.0)

                v_stage = vst.tile([128, LT, D], f32, tag="vstage")
                nc.sync.dma_start(
                    out=v_stage, in_=v_d.ap()[b].rearrange("(t p) d -> p t d", p=128))
                vr = vrp.tile([128, LT, D + 1], fp16, tag="vr")
                nc.vector.tensor_copy(out=vr[:, :, 0:D], in_=v_stage)
                nc.vector.memset(vr[:, :, D:D + 1], 1.0)

                qkT[b] = (qhT, khT3)
                v_r[b] = vr

                jobs = []
                for lt in range(LT):
                    def q_tr_job(lt=lt):
                        tp = trps.tile([64, 128], fp16, tag="tr")
                        nc.tensor.transpose(tp, qh_nat[:, lt, :], identh)
                        nc.vector.tensor_copy(
                            out=qhT[0:64, lt * 128:(lt + 1) * 128], in_=tp)

                    def k_tr_job(lt=lt):
                        tp = trps.tile([64, 128], fp16, tag="tr")
                        nc.tensor.transpose(tp, kh_nat[:, lt, :], identh)
                        h = slice(0, 64) if lt % 2 == 0 else slice(64, 128)
                        nc.vector.tensor_copy(out=khT3[h, lt, :], in_=tp)

                    jobs.append(q_tr_job)
                    jobs.append(k_tr_job)

                def qdup_job():
                    nc.sync.dma_start(out=qhT[64:128, :], in_=qhT[0:64, :])

                jobs.append(qdup_job)
                return jobs

            pending = []   # deferred small jobs woven into the MM stream

            def main(b, next_jobs, weave=2):
                qhT, khT3 = qkT.pop(b)
                vr = v_r.pop(b)
                slot = 0
                for qh in range(NQH):
                    q0 = qh * QHW
                    pv = pvps.tile([D + 1, QHW], f32, tag="pv")

                    for kp in range(LT // 2):      # pairs of k-tiles
                        ka, kb = 2 * kp, 2 * kp + 1
                        # interleave deferred out-work + next batch's prep
                        if pending:
                            pending.pop(0)()
                        for _ in range(weave):
                            if slot < len(next_jobs):
                                next_jobs[slot]()
                                slot += 1
                        s_a = sps.tile([128, QHW], f32, tag="s")
                        s_b = sps.tile([128, QHW], f32, tag="s")
                        # full-C (128) matmuls against the zero-padded
                        # K^T blocks; rhs spans both duplicated halves
                        for s_ps, kt in ((s_a, ka), (s_b, kb)):
                            for j in range(QHW // 512):
                                js = slice(j * 512, (j + 1) * 512)
                                qs = slice(q0 + j * 512, q0 + (j + 1) * 512)
                                nc.tensor.matmul(
                                    s_ps[:, js], khT3[:, kt, :], qhT[:, qs],
                                    start=True, stop=True)
                        for kt, s_ps in ((ka, s_a), (kb, s_b)):
                            e_r = erp.tile([128, QHW], fp16, tag="e")
                            nc.scalar.activation(out=e_r, in_=s_ps, func=Exp,
                                                 scale=float(SCALE))
                            for j in range(QHW // 512):
                                js = slice(j * 512, (j + 1) * 512)
                                nc.tensor.matmul(
                                    pv[:, js], vr[:, kt, :], e_r[:, js],
                                    start=(kt == 0), stop=(kt == LT - 1))

                    # defer psum evacuation + out-transpose + normalize:
                    # woven into subsequent pair-slots so the in-order PE
                    # stream never blocks on this at the qh boundary
                    pv_sb = pvsb.tile([D + 1, QHW], f32, tag="pvsb")
                    o_all = oallp.tile([128, QHW // 128, D], f32, tag="oall")

                    def evac_job(pv=pv, pv_sb=pv_sb):
                        nc.vector.tensor_copy(out=pv_sb, in_=pv)
                    pending.append(evac_job)

                    for qt in range(QHW // 128):
                        def out_job(qt=qt, pv_sb=pv_sb, o_all=o_all):
                            ot = trps.tile([128, D + 1], f32, tag="tr")
                            nc.tensor.transpose(
                                ot, pv_sb[:, qt * 128:(qt + 1) * 128],
                                ident[0:D + 1, 0:D + 1])
                            rz = rzp.tile([128, 1], f32, tag="rz")
                            nc.vector.reciprocal(out=rz, in_=ot[:, D:D + 1])
                            nc.vector.tensor_scalar_mul(
                                out=o_all[:, qt, :], in0=ot[:, 0:D],
                                scalar1=rz)
                        pending.append(out_job)

                    def store_job(b=b, q0=q0, o_all=o_all):
                        nc.sync.dma_start(
                            out=o_d.ap()[b, q0:q0 + QHW, :].rearrange(
                                "(t p) d -> p t d", p=128),
                            in_=o_all)
                    pending.append(store_job)
                while slot < len(next_jobs):
                    next_jobs[slot]()
                    slot += 1

            for _ in range(12):
                warmer()
            jobs0 = prep_load(0)
            tr0 = jobs0[:2 * LT]
            # fast start: inline only what main(0) qh=0 kp=0 needs --
            # Q tiles 0-7 (+ region dup), K tiles 0-1
            for job in [tr0[1], tr0[3]] + [tr0[2 * l] for l in range(8)]:
                job()
            qhT0, khT3_0 = qkT[0]
            nc.sync.dma_start(out=qhT0[64:128, 0:QHW], in_=qhT0[0:64, 0:QHW])

            def region_qdup(lo, hi):
                def job():
                    nc.sync.dma_start(out=qhT0[64:128, lo:hi], in_=qhT0[0:64, lo:hi])
                return job

            # remaining K tiles in consumption order, then Q tiles 8-15
            # for qh=1 plus their upper-half dup
            rest = []
            for l in range(2, LT):
                rest.append(tr0[2 * l + 1])
            rest += [tr0[2 * l] for l in range(8, LT)]
            rest.append(region_qdup(QHW, L))
            for b in range(B_SH):
                nxt = prep_load(b + 1) if b + 1 < B_SH else []
                if b == 0:
                    main(b, rest + nxt, weave=5)
                else:
                    main(b, nxt)
            for job in pending:
                job()

    nc.finalize()
    return nc


def _get_nc():
    if "nc" not in _cached:
        _cached["nc"] = _build()
    return _cached["nc"]


def kernel(query, key, value):
    from concourse.bass_utils import run_bass_kernel_spmd

    nc = _get_nc()
    query = np.ascontiguousarray(query, dtype=np.float32)
    key = np.ascontiguousarray(key, dtype=np.float32)
    value = np.ascontiguousarray(value, dtype=np.float32)

    in_maps = []
    for c in range(NCORES):
        sl = slice(c * B_SH, (c + 1) * B_SH)
        in_maps.append({
            "query": query[sl], "key": key[sl], "value": value[sl]})

    res = run_bass_kernel_spmd(nc, in_maps, core_ids=list(range(NCORES)))
    out = np.concatenate([r["out"] for r in res.results], axis=0)
    return out
